# revision 8
# baseline (speedup 1.0000x reference)
"""Trainium2 kernel for the FEM kinematic (strain) layer.

Reference computation:
    disp = inputs[:, elem_nodes]                      # [B, E, 8, 2]
    dd   = einsum('egkl,bekn->begnl', shpdx, disp)    # [B, E, 9, 2, 2]
    out  = stack([dd[...,0,0], dd[...,1,1],
                  0.5*(dd[...,0,1] + dd[...,1,0])])   # [B, E*9, 3]

Strategy: elements split across 8 NeuronCores.  The host resolves the
element->node indirection and ships fp16 per-element blocks in a
partition-major layout.  On the device the per-element contraction over the
8 nodes runs on the TensorEngine: 16 elements per round are packed as 8x8
blocks on the diagonal of the stationary operand (built on the VectorEngine
as broadcast*mask in one fp16 2x op), and tile_position matmuls compute
    dd[(el,b,n), (g,l)] = sum_k disp[el][k,(b,n)] * shpdx[el][(g,l),k]
into PSUM.  Banks alternate between 4x(32x32) and 2x(64x64) subarray
splits so the VectorE (mask build, cost ~ W columns) and TensorE (cost ~
output columns) loads average out below the DMA roofline.  Full PSUM banks
(28 rounds = 448 elements) are evicted once as fp16 by the ScalarEngine and
DMA'd out; the host unpacks dd and combines the strain components.
"""

import sys

import numpy as np

sys.path.insert(0, "/opt/trn_rl_repo")

import concourse.bacc as bacc
import concourse.mybir as mybir
import concourse.tile as tile
from concourse.bass_utils import run_bass_kernel_spmd

B = 4
N_NODES = 1_000_000
N_ELEM = 500_000
N_GP = 9
N_EN = 8
N_CORES = 8

P = 128
NR = 28                    # rounds (16 elements each) per PSUM bank
EPB = 16 * NR              # elements per bank = 448
NBANK = 140                # banks per core
E_CORE = N_ELEM // N_CORES      # 62500
E_PAD = NBANK * EPB             # 62720
G = 10                     # banks per input DMA group
EG = 4                     # banks per output DMA group
NBG = NBANK // G           # 14
NEVG = NBANK // EG         # 35


def _is_sub2(bk: int) -> bool:
    return bk % 2 == 0


_compiled = None


def _build_program():
    nc = bacc.Bacc("TRN2", target_bir_lowering=False, debug=False)
    f16 = mybir.dt.float16
    f32 = mybir.dt.float32

    # disp, dense:  [bg, p=(grp,el,k), gi, r, (b,n)]
    d_d = nc.dram_tensor("d_in", [NBG, P, G, NR, 8], f16, kind="ExternalInput").ap()
    # shpdx, rhs-ready: [bg, p=(grp,el,k), gi, r, (g,l)]
    s_d = nc.dram_tensor("s_in", [NBG, P, G, NR, 18], f16, kind="ExternalInput").ap()
    # block-diag masks, packed: cols 0:32 sub4, cols 32:96 sub2
    m_d = nc.dram_tensor("mask", [P, 96], f16, kind="ExternalInput").ap()
    # dd out: [ev, p=(grp, el', b, n), eslot, r, (g,l)]
    o_d = nc.dram_tensor("out", [NEVG, P, EG, NR, 18], f16, kind="ExternalOutput").ap()

    with tile.TileContext(nc) as tc:
        with (
            tc.tile_pool(name="const", bufs=1) as const_pool,
            tc.tile_pool(name="io", bufs=3) as io_pool,
            tc.tile_pool(name="w", bufs=4) as w_pool,
            tc.tile_pool(name="ps", bufs=6, space="PSUM") as ps_pool,
            tc.tile_pool(name="ev", bufs=3) as ev_pool,
        ):
            Mt = const_pool.tile([P, 96], f16, tag="M")
            nc.sync.dma_start(out=Mt[:], in_=m_d)
            Mv = Mt[:, :32].rearrange("p (e b) -> p e b", e=4)
            M2v = Mt[:, 32:].rearrange("p (e b) -> p e b", e=8)

            ev = None
            for bg in range(NBG):
                D = io_pool.tile([P, G * NR * 8], f16, tag="D")
                S = io_pool.tile([P, G * NR * 18], f16, tag="S")
                nc.sync.dma_start(out=D[:], in_=d_d[bg].rearrange("p g r b -> p (g r b)"))
                nc.sync.dma_start(out=S[:], in_=s_d[bg].rearrange("p g r b -> p (g r b)"))

                for gi in range(G):
                    bk = bg * G + gi
                    if bk % EG == 0:
                        ev = ev_pool.tile([P, EG * NR * 18], f16, tag="ev")

                    Dv = D[:].rearrange("p (g r b) -> p g r b", g=G, r=NR)
                    ps = ps_pool.tile([P, 512], f32, tag="ps")

                    if _is_sub2(bk):
                        # W[p, (r, el', bn)] = D[p, (gi, r, bn)] * M2[p, (el', bn)]
                        W = w_pool.tile([P, NR * 64], f16, tag="W2")
                        Wv = W[:].rearrange("p (r e b) -> p r e b", r=NR, e=8)
                        Db = Dv[:, gi, :, None, :].to_broadcast([P, NR, 8, 8])
                        Mb = M2v[:, None, :, :].to_broadcast([P, NR, 8, 8])
                        nc.vector.tensor_tensor(out=Wv, in0=Db, in1=Mb,
                                                op=mybir.AluOpType.mult)
                        for r in range(NR):
                            for h in range(2):
                                pr = slice(64 * h, 64 * h + 64)
                                nc.tensor.matmul(
                                    out=ps[pr, r * 18:(r + 1) * 18],
                                    lhsT=W[pr, r * 64:(r + 1) * 64],
                                    rhs=S[pr, (gi * NR + r) * 18:(gi * NR + r + 1) * 18],
                                    start=True, stop=True,
                                    tile_position=(64 * h, 64 * h),
                                )
                    else:
                        W = w_pool.tile([P, NR * 32], f16, tag="W")
                        Wv = W[:].rearrange("p (r e b) -> p r e b", r=NR, e=4)
                        Db = Dv[:, gi, :, None, :].to_broadcast([P, NR, 4, 8])
                        Mb = Mv[:, None, :, :].to_broadcast([P, NR, 4, 8])
                        nc.vector.tensor_tensor(out=Wv, in0=Db, in1=Mb,
                                                op=mybir.AluOpType.mult)
                        for r in range(NR):
                            for i in range(4):
                                pr = slice(32 * i, 32 * i + 32)
                                nc.tensor.matmul(
                                    out=ps[pr, r * 18:(r + 1) * 18],
                                    lhsT=W[pr, r * 32:(r + 1) * 32],
                                    rhs=S[pr, (gi * NR + r) * 18:(gi * NR + r + 1) * 18],
                                    start=True, stop=True,
                                    tile_position=(32 * i, 32 * i),
                                )

                    eslot = bk % EG
                    nc.scalar.copy(
                        out=ev[:, eslot * NR * 18:(eslot + 1) * NR * 18],
                        in_=ps[:, :NR * 18])

                    if eslot == EG - 1:
                        nc.sync.dma_start(
                            out=o_d[bk // EG].rearrange("p e r g -> p (e r g)"),
                            in_=ev[:])

    nc.compile()
    return nc


def _get_program():
    global _compiled
    if _compiled is None:
        _compiled = _build_program()
    return _compiled


def _make_masks():
    m = np.zeros((P, 96), np.float16)
    for p in range(P):
        el4 = (p % 32) // 8
        m[p, el4 * 8:(el4 + 1) * 8] = 1.0
        el8 = (p % 64) // 8
        m[p, 32 + el8 * 8:32 + (el8 + 1) * 8] = 1.0
    return m


_SUB2_BANKS = np.array([bk for bk in range(NBANK) if _is_sub2(bk)])
_SUB4_BANKS = np.array([bk for bk in range(NBANK) if not _is_sub2(bk)])


def _marshal_core(inputs_f16: np.ndarray, shpdx: np.ndarray,
                  elem_nodes: np.ndarray, c: int):
    """Build the d_in / s_in arrays for core c."""
    sl = slice(c * E_CORE, (c + 1) * E_CORE)
    en = elem_nodes[sl]                                   # [E, 8]
    disp = inputs_f16[:, en]                              # [B, E, 8, 2] f16
    # -> [E, k, (b, n)]
    dispc = np.ascontiguousarray(disp.transpose(1, 2, 0, 3)).reshape(E_CORE, 8, 8)
    dpad = np.zeros((E_PAD, 8, 8), np.float16)
    dpad[:E_CORE] = dispc
    spad = np.zeros((E_PAD, N_GP, 8, 2), np.float16)
    spad[:E_CORE] = shpdx[sl].astype(np.float16)

    # e = bk*EPB + r*16 + grp*per + el ; partition p = grp*per*8 + el*8 + k
    d_all = np.empty((NBANK, P, NR, 8), np.float16)
    s_all = np.empty((NBANK, P, NR, 18), np.float16)
    db = dpad.reshape(NBANK, NR, 16, 8, 8)                # bk r sub k bn
    sb = spad.reshape(NBANK, NR, 16, N_GP, 8, 2)          # bk r sub g k l
    for banks, ngrp, per in ((_SUB2_BANKS, 2, 8), (_SUB4_BANKS, 4, 4)):
        dv = db[banks].reshape(len(banks), NR, ngrp, per, 8, 8)
        dv = dv.transpose(0, 2, 3, 4, 1, 5)               # bk grp el k r bn
        d_all[banks] = dv.reshape(len(banks), P, NR, 8)
        sv = sb[banks].reshape(len(banks), NR, ngrp, per, N_GP, 8, 2)
        sv = sv.transpose(0, 2, 3, 5, 1, 4, 6)            # bk grp el k r g l
        s_all[banks] = sv.reshape(len(banks), P, NR, 18)

    d_in = np.ascontiguousarray(
        d_all.reshape(NBG, G, P, NR, 8).transpose(0, 2, 1, 3, 4))
    s_in = np.ascontiguousarray(
        s_all.reshape(NBG, G, P, NR, 18).transpose(0, 2, 1, 3, 4))
    return d_in, s_in


def _decode_core(o: np.ndarray) -> np.ndarray:
    """o: [NEVG, P, EG, NR, 18] fp16 -> strains [B, E_CORE*9, 3] f32."""
    # out partition p = grp*per*8 + el'*8 + (b*2+n)
    ob = o.reshape(NEVG, P, EG, NR, N_GP, 2).transpose(0, 2, 1, 3, 4, 5)
    ob = ob.reshape(NBANK, P, NR, N_GP, 2)                # bk p r g l
    dd = np.empty((B, E_PAD, N_GP, 2, 2), np.float16)
    ddv = dd.reshape(B, NBANK, NR, 16, N_GP, 2, 2)
    for banks, ngrp, per in ((_SUB2_BANKS, 2, 8), (_SUB4_BANKS, 4, 4)):
        ov = ob[banks].reshape(len(banks), ngrp, per, B, 2, NR, N_GP, 2)
        # -> b bk r (grp el) g n l
        ov = ov.transpose(3, 0, 5, 1, 2, 6, 4, 7)
        ddv[:, banks] = ov.reshape(B, len(banks), NR, 16, N_GP, 2, 2)
    dd = dd[:, :E_CORE].astype(np.float32)
    e_xx = dd[..., 0, 0]
    e_yy = dd[..., 1, 1]
    e_xy = 0.5 * (dd[..., 0, 1] + dd[..., 1, 0])
    e = np.stack([e_xx, e_yy, e_xy], axis=-1)             # [B, E, 9, 3]
    return e.reshape(B, E_CORE * N_GP, 3)


def kernel(inputs, shpdx, elem_nodes, _want_trace=False):
    nc = _get_program()

    inputs_f16 = inputs.astype(np.float16)
    mask = _make_masks()
    in_maps = []
    for c in range(N_CORES):
        d_in, s_in = _marshal_core(inputs_f16, shpdx, elem_nodes, c)
        in_maps.append({"d_in": d_in, "s_in": s_in, "mask": mask})

    core_ids = list(range(N_CORES))
    res = run_bass_kernel_spmd(nc, in_maps, core_ids, trace=_want_trace)

    outs = []
    for c in range(N_CORES):
        outs.append(_decode_core(np.asarray(res.results[c]["out"])))
    full = np.concatenate(outs, axis=1)                   # [B, N_ELEM*9, 3]
    if _want_trace:
        return full, res
    return full


# revision 10
# speedup vs baseline: 1.1304x; 1.1304x over previous
"""Trainium2 kernel for the FEM kinematic (strain) layer.

Reference computation:
    disp = inputs[:, elem_nodes]                      # [B, E, 8, 2]
    dd   = einsum('egkl,bekn->begnl', shpdx, disp)    # [B, E, 9, 2, 2]
    out  = stack([dd[...,0,0], dd[...,1,1],
                  0.5*(dd[...,0,1] + dd[...,1,0])])   # [B, E*9, 3]

Strategy: elements split across 8 NeuronCores.  The host resolves the
element->node indirection and ships fp16 per-element blocks in a
partition-major layout.  On the device the per-element contraction over the
8 nodes runs on the TensorEngine: 16 elements per round are packed as 8x8
blocks on the diagonal of the stationary operand (built on the VectorEngine
as broadcast*mask in one fp16 2x op), and tile_position matmuls compute
    dd[(el,b,n), (g,l)] = sum_k disp[el][k,(b,n)] * shpdx[el][(g,l),k]
into PSUM.  Banks alternate between 4x(32x32) and 2x(64x64) subarray
splits so the VectorE (mask build, cost ~ W columns) and TensorE (cost ~
output columns) loads average out below the DMA roofline.  Full PSUM banks
(28 rounds = 448 elements) are evicted once as fp16 by the ScalarEngine and
DMA'd out; the host unpacks dd and combines the strain components.
"""

import sys

import numpy as np

sys.path.insert(0, "/opt/trn_rl_repo")

import concourse.bacc as bacc
import concourse.mybir as mybir
import concourse.tile as tile
from concourse.bass_utils import run_bass_kernel_spmd

B = 4
N_NODES = 1_000_000
N_ELEM = 500_000
N_GP = 9
N_EN = 8
N_CORES = 8

P = 128
NR = 28                    # rounds (16 elements each) per PSUM bank
EPB = 16 * NR              # elements per bank = 448
NBANK = 140                # banks per core
E_CORE = N_ELEM // N_CORES      # 62500
E_PAD = NBANK * EPB             # 62720
G = 10                     # banks per input DMA group
EG = 4                     # banks per output DMA group
NBG = NBANK // G           # 14
NEVG = NBANK // EG         # 35


def _is_sub2(bk: int) -> bool:
    return bk % 5 < 2


_compiled = None


def _build_program():
    nc = bacc.Bacc("TRN2", target_bir_lowering=False, debug=False)
    f16 = mybir.dt.float16
    f32 = mybir.dt.float32

    # disp, dense:  [bg, p=(grp,el,k), gi, r, (b,n)]
    d_d = nc.dram_tensor("d_in", [NBG, P, G, NR, 8], f16, kind="ExternalInput").ap()
    # shpdx, rhs-ready: [bg, p=(grp,el,k), gi, r, (g,l)]
    s_d = nc.dram_tensor("s_in", [NBG, P, G, NR, 18], f16, kind="ExternalInput").ap()
    # block-diag masks, packed: cols 0:32 sub4, cols 32:96 sub2
    m_d = nc.dram_tensor("mask", [P, 96], f16, kind="ExternalInput").ap()
    # zero ctx-index vector for the writeback stores
    z_d = nc.dram_tensor("zidx", [P, 1], mybir.dt.int32, kind="ExternalInput").ap()
    # dd out: [ev, 1, p=(grp, el', b, n), 1, (eslot, r, (g,l)) padded to 2048]
    o_d = nc.dram_tensor("out", [NEVG, 1, P, 1, 2048], f16, kind="ExternalOutput").ap()

    with tile.TileContext(nc) as tc:
        with (
            tc.tile_pool(name="const", bufs=1) as const_pool,
            tc.tile_pool(name="io", bufs=3) as io_pool,
            tc.tile_pool(name="w", bufs=4) as w_pool,
            tc.tile_pool(name="ps", bufs=6, space="PSUM") as ps_pool,
            tc.tile_pool(name="ev", bufs=3) as ev_pool,
        ):
            Mt = const_pool.tile([P, 96], f16, tag="M")
            nc.sync.dma_start(out=Mt[:], in_=m_d)
            Zt = const_pool.tile([P, 1], mybir.dt.int32, tag="Z")
            nc.sync.dma_start(out=Zt[:], in_=z_d)
            Mv = Mt[:, :32].rearrange("p (e b) -> p e b", e=4)
            M2v = Mt[:, 32:].rearrange("p (e b) -> p e b", e=8)

            ev = None
            for bg in range(NBG):
                D = io_pool.tile([P, G * NR * 8], f16, tag="D")
                S = io_pool.tile([P, G * NR * 18], f16, tag="S")
                nc.sync.dma_start(out=D[:], in_=d_d[bg].rearrange("p g r b -> p (g r b)"))
                nc.sync.dma_start(out=S[:], in_=s_d[bg].rearrange("p g r b -> p (g r b)"))

                for gi in range(G):
                    bk = bg * G + gi
                    if bk % EG == 0:
                        ev = ev_pool.tile([P, 2048], f16, tag="ev")
                        nc.vector.memset(ev[:, EG * NR * 18:], 0.0)

                    Dv = D[:].rearrange("p (g r b) -> p g r b", g=G, r=NR)
                    ps = ps_pool.tile([P, 512], f32, tag="ps")

                    if _is_sub2(bk):
                        # W[p, (r, el', bn)] = D[p, (gi, r, bn)] * M2[p, (el', bn)]
                        W = w_pool.tile([P, NR * 64], f16, tag="W2")
                        Wv = W[:].rearrange("p (r e b) -> p r e b", r=NR, e=8)
                        Db = Dv[:, gi, :, None, :].to_broadcast([P, NR, 8, 8])
                        Mb = M2v[:, None, :, :].to_broadcast([P, NR, 8, 8])
                        nc.vector.tensor_tensor(out=Wv, in0=Db, in1=Mb,
                                                op=mybir.AluOpType.mult)
                        for r in range(NR):
                            for h in range(2):
                                pr = slice(64 * h, 64 * h + 64)
                                nc.tensor.matmul(
                                    out=ps[pr, r * 18:(r + 1) * 18],
                                    lhsT=W[pr, r * 64:(r + 1) * 64],
                                    rhs=S[pr, (gi * NR + r) * 18:(gi * NR + r + 1) * 18],
                                    start=True, stop=True,
                                    tile_position=(64 * h, 64 * h),
                                )
                    else:
                        W = w_pool.tile([P, NR * 32], f16, tag="W")
                        Wv = W[:].rearrange("p (r e b) -> p r e b", r=NR, e=4)
                        Db = Dv[:, gi, :, None, :].to_broadcast([P, NR, 4, 8])
                        Mb = Mv[:, None, :, :].to_broadcast([P, NR, 4, 8])
                        nc.vector.tensor_tensor(out=Wv, in0=Db, in1=Mb,
                                                op=mybir.AluOpType.mult)
                        for r in range(NR):
                            for i in range(4):
                                pr = slice(32 * i, 32 * i + 32)
                                nc.tensor.matmul(
                                    out=ps[pr, r * 18:(r + 1) * 18],
                                    lhsT=W[pr, r * 32:(r + 1) * 32],
                                    rhs=S[pr, (gi * NR + r) * 18:(gi * NR + r + 1) * 18],
                                    start=True, stop=True,
                                    tile_position=(32 * i, 32 * i),
                                )

                    eslot = bk % EG
                    nc.scalar.copy(
                        out=ev[:, eslot * NR * 18:(eslot + 1) * NR * 18],
                        in_=ps[:, :NR * 18])

                    if eslot == EG - 1:
                        nc.gpsimd.kv_writeback(
                            out_ap=o_d[bk // EG],
                            in_ap=ev[:].rearrange("p (a b n) -> p a b n",
                                                  a=1, b=1),
                            ctx_idxs_ap=Zt[:])

    nc.compile()
    return nc


def _get_program():
    global _compiled
    if _compiled is None:
        _compiled = _build_program()
    return _compiled


def _make_masks():
    m = np.zeros((P, 96), np.float16)
    for p in range(P):
        el4 = (p % 32) // 8
        m[p, el4 * 8:(el4 + 1) * 8] = 1.0
        el8 = (p % 64) // 8
        m[p, 32 + el8 * 8:32 + (el8 + 1) * 8] = 1.0
    return m


_SUB2_BANKS = np.array([bk for bk in range(NBANK) if _is_sub2(bk)])
_SUB4_BANKS = np.array([bk for bk in range(NBANK) if not _is_sub2(bk)])


def _marshal_core(inputs_f16: np.ndarray, shpdx: np.ndarray,
                  elem_nodes: np.ndarray, c: int):
    """Build the d_in / s_in arrays for core c."""
    sl = slice(c * E_CORE, (c + 1) * E_CORE)
    en = elem_nodes[sl]                                   # [E, 8]
    disp = inputs_f16[:, en]                              # [B, E, 8, 2] f16
    # -> [E, k, (b, n)]
    dispc = np.ascontiguousarray(disp.transpose(1, 2, 0, 3)).reshape(E_CORE, 8, 8)
    dpad = np.zeros((E_PAD, 8, 8), np.float16)
    dpad[:E_CORE] = dispc
    spad = np.zeros((E_PAD, N_GP, 8, 2), np.float16)
    spad[:E_CORE] = shpdx[sl].astype(np.float16)

    # e = bk*EPB + r*16 + grp*per + el ; partition p = grp*per*8 + el*8 + k
    d_all = np.empty((NBANK, P, NR, 8), np.float16)
    s_all = np.empty((NBANK, P, NR, 18), np.float16)
    db = dpad.reshape(NBANK, NR, 16, 8, 8)                # bk r sub k bn
    sb = spad.reshape(NBANK, NR, 16, N_GP, 8, 2)          # bk r sub g k l
    for banks, ngrp, per in ((_SUB2_BANKS, 2, 8), (_SUB4_BANKS, 4, 4)):
        dv = db[banks].reshape(len(banks), NR, ngrp, per, 8, 8)
        dv = dv.transpose(0, 2, 3, 4, 1, 5)               # bk grp el k r bn
        d_all[banks] = dv.reshape(len(banks), P, NR, 8)
        sv = sb[banks].reshape(len(banks), NR, ngrp, per, N_GP, 8, 2)
        sv = sv.transpose(0, 2, 3, 5, 1, 4, 6)            # bk grp el k r g l
        s_all[banks] = sv.reshape(len(banks), P, NR, 18)

    d_in = np.ascontiguousarray(
        d_all.reshape(NBG, G, P, NR, 8).transpose(0, 2, 1, 3, 4))
    s_in = np.ascontiguousarray(
        s_all.reshape(NBG, G, P, NR, 18).transpose(0, 2, 1, 3, 4))
    return d_in, s_in


def _decode_core(o: np.ndarray) -> np.ndarray:
    """o: [NEVG, P, EG, NR, 18] fp16 -> strains [B, E_CORE*9, 3] f32."""
    # out partition p = grp*per*8 + el'*8 + (b*2+n)
    o = o[:, 0, :, 0, :EG * NR * 18]
    ob = o.reshape(NEVG, P, EG, NR, N_GP, 2).transpose(0, 2, 1, 3, 4, 5)
    ob = ob.reshape(NBANK, P, NR, N_GP, 2)                # bk p r g l
    dd = np.empty((B, E_PAD, N_GP, 2, 2), np.float16)
    ddv = dd.reshape(B, NBANK, NR, 16, N_GP, 2, 2)
    for banks, ngrp, per in ((_SUB2_BANKS, 2, 8), (_SUB4_BANKS, 4, 4)):
        ov = ob[banks].reshape(len(banks), ngrp, per, B, 2, NR, N_GP, 2)
        # -> b bk r (grp el) g n l
        ov = ov.transpose(3, 0, 5, 1, 2, 6, 4, 7)
        ddv[:, banks] = ov.reshape(B, len(banks), NR, 16, N_GP, 2, 2)
    dd = dd[:, :E_CORE].astype(np.float32)
    e_xx = dd[..., 0, 0]
    e_yy = dd[..., 1, 1]
    e_xy = 0.5 * (dd[..., 0, 1] + dd[..., 1, 0])
    e = np.stack([e_xx, e_yy, e_xy], axis=-1)             # [B, E, 9, 3]
    return e.reshape(B, E_CORE * N_GP, 3)


def kernel(inputs, shpdx, elem_nodes, _want_trace=False):
    nc = _get_program()

    inputs_f16 = inputs.astype(np.float16)
    mask = _make_masks()
    in_maps = []
    for c in range(N_CORES):
        d_in, s_in = _marshal_core(inputs_f16, shpdx, elem_nodes, c)
        in_maps.append({"d_in": d_in, "s_in": s_in, "mask": mask,
                        "zidx": np.zeros((P, 1), np.int32)})

    core_ids = list(range(N_CORES))
    res = run_bass_kernel_spmd(nc, in_maps, core_ids, trace=_want_trace)

    outs = []
    for c in range(N_CORES):
        outs.append(_decode_core(np.asarray(res.results[c]["out"])))
    full = np.concatenate(outs, axis=1)                   # [B, N_ELEM*9, 3]
    if _want_trace:
        return full, res
    return full


# revision 11
# speedup vs baseline: 1.1380x; 1.0067x over previous
"""Trainium2 kernel for the FEM kinematic (strain) layer.

Reference computation:
    disp = inputs[:, elem_nodes]                      # [B, E, 8, 2]
    dd   = einsum('egkl,bekn->begnl', shpdx, disp)    # [B, E, 9, 2, 2]
    out  = stack([dd[...,0,0], dd[...,1,1],
                  0.5*(dd[...,0,1] + dd[...,1,0])])   # [B, E*9, 3]

Strategy: elements split across 8 NeuronCores.  The host resolves the
element->node indirection and ships fp16 per-element blocks in a
partition-major layout.  On the device the per-element contraction over the
8 nodes runs on the TensorEngine: 16 elements per round are packed as 8x8
blocks on the diagonal of the stationary operand (built on the VectorEngine
as broadcast*mask in one fp16 2x op), and tile_position matmuls compute
    dd[(el,b,n), (g,l)] = sum_k disp[el][k,(b,n)] * shpdx[el][(g,l),k]
into PSUM.  Banks alternate between 4x(32x32) and 2x(64x64) subarray
splits so the VectorE (mask build, cost ~ W columns) and TensorE (cost ~
output columns) loads average out below the DMA roofline.  Full PSUM banks
(28 rounds = 448 elements) are evicted once as fp16 by the ScalarEngine and
DMA'd out; the host unpacks dd and combines the strain components.
"""

import sys

import numpy as np

sys.path.insert(0, "/opt/trn_rl_repo")

import concourse.bacc as bacc
import concourse.mybir as mybir
import concourse.tile as tile
from concourse.bass_utils import run_bass_kernel_spmd

B = 4
N_NODES = 1_000_000
N_ELEM = 500_000
N_GP = 9
N_EN = 8
N_CORES = 8

P = 128
NR = 28                    # rounds (16 elements each) per PSUM bank
EPB = 16 * NR              # elements per bank = 448
NBANK = 140                # banks per core
E_CORE = N_ELEM // N_CORES      # 62500
E_PAD = NBANK * EPB             # 62720
G = 10                     # banks per input DMA group
EG = 4                     # banks per output DMA group
NBG = NBANK // G           # 14
NEVG = NBANK // EG         # 35


def _is_sub2(bk: int) -> bool:
    return bk % 5 < 2


_compiled = None


def _build_program():
    nc = bacc.Bacc("TRN2", target_bir_lowering=False, debug=False)
    f16 = mybir.dt.float16
    f32 = mybir.dt.float32

    # disp, dense:  [bg, p=(grp,el,k), gi, r, (b,n)]
    d_d = nc.dram_tensor("d_in", [NBG, P, G, NR, 8], f16, kind="ExternalInput").ap()
    # shpdx, rhs-ready: [bg, p=(grp,el,k), gi, r, (g,l)]
    s_d = nc.dram_tensor("s_in", [NBG, P, G, NR, 18], f16, kind="ExternalInput").ap()
    # block-diag masks, packed: cols 0:32 sub4, cols 32:96 sub2
    m_d = nc.dram_tensor("mask", [P, 96], f16, kind="ExternalInput").ap()
    # zero ctx-index vector for the writeback stores
    z_d = nc.dram_tensor("zidx", [P, 1], mybir.dt.int32, kind="ExternalInput").ap()
    # dd out: [ev, 1, p=(grp, el', b, n), 1, (eslot, r, (g,l)) padded to 2048]
    o_d = nc.dram_tensor("out", [NEVG, 1, P, 1, 2048], f16, kind="ExternalOutput").ap()

    with tile.TileContext(nc) as tc:
        with (
            tc.tile_pool(name="const", bufs=1) as const_pool,
            tc.tile_pool(name="io", bufs=3) as io_pool,
            tc.tile_pool(name="w", bufs=4) as w_pool,
            tc.tile_pool(name="ps", bufs=6, space="PSUM") as ps_pool,
            tc.tile_pool(name="ev", bufs=6) as ev_pool,
        ):
            Mt = const_pool.tile([P, 96], f16, tag="M")
            nc.sync.dma_start(out=Mt[:], in_=m_d)
            Zt = const_pool.tile([P, 1], mybir.dt.int32, tag="Z")
            nc.sync.dma_start(out=Zt[:], in_=z_d)
            Mv = Mt[:, :32].rearrange("p (e b) -> p e b", e=4)
            M2v = Mt[:, 32:].rearrange("p (e b) -> p e b", e=8)

            ev = None
            for bg in range(NBG):
                D = io_pool.tile([P, G * NR * 8], f16, tag="D")
                S = io_pool.tile([P, G * NR * 18], f16, tag="S")
                nc.sync.dma_start(out=D[:], in_=d_d[bg].rearrange("p g r b -> p (g r b)"))
                nc.sync.dma_start(out=S[:], in_=s_d[bg].rearrange("p g r b -> p (g r b)"))

                for gi in range(G):
                    bk = bg * G + gi
                    if bk % EG == 0:
                        ev = ev_pool.tile([P, 2048], f16, tag="ev")
                        nc.vector.memset(ev[:, EG * NR * 18:], 0.0)

                    Dv = D[:].rearrange("p (g r b) -> p g r b", g=G, r=NR)
                    ps = ps_pool.tile([P, 512], f32, tag="ps")

                    if _is_sub2(bk):
                        # W[p, (r, el', bn)] = D[p, (gi, r, bn)] * M2[p, (el', bn)]
                        W = w_pool.tile([P, NR * 64], f16, tag="W2")
                        Wv = W[:].rearrange("p (r e b) -> p r e b", r=NR, e=8)
                        Db = Dv[:, gi, :, None, :].to_broadcast([P, NR, 8, 8])
                        Mb = M2v[:, None, :, :].to_broadcast([P, NR, 8, 8])
                        nc.vector.tensor_tensor(out=Wv, in0=Db, in1=Mb,
                                                op=mybir.AluOpType.mult)
                        for r in range(NR):
                            for h in range(2):
                                pr = slice(64 * h, 64 * h + 64)
                                nc.tensor.matmul(
                                    out=ps[pr, r * 18:(r + 1) * 18],
                                    lhsT=W[pr, r * 64:(r + 1) * 64],
                                    rhs=S[pr, (gi * NR + r) * 18:(gi * NR + r + 1) * 18],
                                    start=True, stop=True,
                                    tile_position=(64 * h, 64 * h),
                                )
                    else:
                        W = w_pool.tile([P, NR * 32], f16, tag="W")
                        Wv = W[:].rearrange("p (r e b) -> p r e b", r=NR, e=4)
                        Db = Dv[:, gi, :, None, :].to_broadcast([P, NR, 4, 8])
                        Mb = Mv[:, None, :, :].to_broadcast([P, NR, 4, 8])
                        nc.vector.tensor_tensor(out=Wv, in0=Db, in1=Mb,
                                                op=mybir.AluOpType.mult)
                        for r in range(NR):
                            for i in range(4):
                                pr = slice(32 * i, 32 * i + 32)
                                nc.tensor.matmul(
                                    out=ps[pr, r * 18:(r + 1) * 18],
                                    lhsT=W[pr, r * 32:(r + 1) * 32],
                                    rhs=S[pr, (gi * NR + r) * 18:(gi * NR + r + 1) * 18],
                                    start=True, stop=True,
                                    tile_position=(32 * i, 32 * i),
                                )

                    eslot = bk % EG
                    nc.scalar.copy(
                        out=ev[:, eslot * NR * 18:(eslot + 1) * NR * 18],
                        in_=ps[:, :NR * 18])

                    if eslot == EG - 1:
                        nc.gpsimd.kv_writeback(
                            out_ap=o_d[bk // EG],
                            in_ap=ev[:].rearrange("p (a b n) -> p a b n",
                                                  a=1, b=1),
                            ctx_idxs_ap=Zt[:])

    nc.compile()
    return nc


def _get_program():
    global _compiled
    if _compiled is None:
        _compiled = _build_program()
    return _compiled


def _make_masks():
    m = np.zeros((P, 96), np.float16)
    for p in range(P):
        el4 = (p % 32) // 8
        m[p, el4 * 8:(el4 + 1) * 8] = 1.0
        el8 = (p % 64) // 8
        m[p, 32 + el8 * 8:32 + (el8 + 1) * 8] = 1.0
    return m


_SUB2_BANKS = np.array([bk for bk in range(NBANK) if _is_sub2(bk)])
_SUB4_BANKS = np.array([bk for bk in range(NBANK) if not _is_sub2(bk)])


def _marshal_core(inputs_f16: np.ndarray, shpdx: np.ndarray,
                  elem_nodes: np.ndarray, c: int):
    """Build the d_in / s_in arrays for core c."""
    sl = slice(c * E_CORE, (c + 1) * E_CORE)
    en = elem_nodes[sl]                                   # [E, 8]
    disp = inputs_f16[:, en]                              # [B, E, 8, 2] f16
    # -> [E, k, (b, n)]
    dispc = np.ascontiguousarray(disp.transpose(1, 2, 0, 3)).reshape(E_CORE, 8, 8)
    dpad = np.zeros((E_PAD, 8, 8), np.float16)
    dpad[:E_CORE] = dispc
    spad = np.zeros((E_PAD, N_GP, 8, 2), np.float16)
    spad[:E_CORE] = shpdx[sl].astype(np.float16)

    # e = bk*EPB + r*16 + grp*per + el ; partition p = grp*per*8 + el*8 + k
    d_all = np.empty((NBANK, P, NR, 8), np.float16)
    s_all = np.empty((NBANK, P, NR, 18), np.float16)
    db = dpad.reshape(NBANK, NR, 16, 8, 8)                # bk r sub k bn
    sb = spad.reshape(NBANK, NR, 16, N_GP, 8, 2)          # bk r sub g k l
    for banks, ngrp, per in ((_SUB2_BANKS, 2, 8), (_SUB4_BANKS, 4, 4)):
        dv = db[banks].reshape(len(banks), NR, ngrp, per, 8, 8)
        dv = dv.transpose(0, 2, 3, 4, 1, 5)               # bk grp el k r bn
        d_all[banks] = dv.reshape(len(banks), P, NR, 8)
        sv = sb[banks].reshape(len(banks), NR, ngrp, per, N_GP, 8, 2)
        sv = sv.transpose(0, 2, 3, 5, 1, 4, 6)            # bk grp el k r g l
        s_all[banks] = sv.reshape(len(banks), P, NR, 18)

    d_in = np.ascontiguousarray(
        d_all.reshape(NBG, G, P, NR, 8).transpose(0, 2, 1, 3, 4))
    s_in = np.ascontiguousarray(
        s_all.reshape(NBG, G, P, NR, 18).transpose(0, 2, 1, 3, 4))
    return d_in, s_in


def _decode_core(o: np.ndarray) -> np.ndarray:
    """o: [NEVG, P, EG, NR, 18] fp16 -> strains [B, E_CORE*9, 3] f32."""
    # out partition p = grp*per*8 + el'*8 + (b*2+n)
    o = o[:, 0, :, 0, :EG * NR * 18]
    ob = o.reshape(NEVG, P, EG, NR, N_GP, 2).transpose(0, 2, 1, 3, 4, 5)
    ob = ob.reshape(NBANK, P, NR, N_GP, 2)                # bk p r g l
    dd = np.empty((B, E_PAD, N_GP, 2, 2), np.float16)
    ddv = dd.reshape(B, NBANK, NR, 16, N_GP, 2, 2)
    for banks, ngrp, per in ((_SUB2_BANKS, 2, 8), (_SUB4_BANKS, 4, 4)):
        ov = ob[banks].reshape(len(banks), ngrp, per, B, 2, NR, N_GP, 2)
        # -> b bk r (grp el) g n l
        ov = ov.transpose(3, 0, 5, 1, 2, 6, 4, 7)
        ddv[:, banks] = ov.reshape(B, len(banks), NR, 16, N_GP, 2, 2)
    dd = dd[:, :E_CORE].astype(np.float32)
    e_xx = dd[..., 0, 0]
    e_yy = dd[..., 1, 1]
    e_xy = 0.5 * (dd[..., 0, 1] + dd[..., 1, 0])
    e = np.stack([e_xx, e_yy, e_xy], axis=-1)             # [B, E, 9, 3]
    return e.reshape(B, E_CORE * N_GP, 3)


def kernel(inputs, shpdx, elem_nodes, _want_trace=False):
    nc = _get_program()

    inputs_f16 = inputs.astype(np.float16)
    mask = _make_masks()
    in_maps = []
    for c in range(N_CORES):
        d_in, s_in = _marshal_core(inputs_f16, shpdx, elem_nodes, c)
        in_maps.append({"d_in": d_in, "s_in": s_in, "mask": mask,
                        "zidx": np.zeros((P, 1), np.int32)})

    core_ids = list(range(N_CORES))
    res = run_bass_kernel_spmd(nc, in_maps, core_ids, trace=_want_trace)

    outs = []
    for c in range(N_CORES):
        outs.append(_decode_core(np.asarray(res.results[c]["out"])))
    full = np.concatenate(outs, axis=1)                   # [B, N_ELEM*9, 3]
    if _want_trace:
        return full, res
    return full


# revision 12
# speedup vs baseline: 1.1518x; 1.0121x over previous
"""Trainium2 kernel for the FEM kinematic (strain) layer.

Reference computation:
    disp = inputs[:, elem_nodes]                      # [B, E, 8, 2]
    dd   = einsum('egkl,bekn->begnl', shpdx, disp)    # [B, E, 9, 2, 2]
    out  = stack([dd[...,0,0], dd[...,1,1],
                  0.5*(dd[...,0,1] + dd[...,1,0])])   # [B, E*9, 3]

Strategy: elements split across 8 NeuronCores.  The host resolves the
element->node indirection and ships fp16 per-element blocks in a
partition-major layout.  On the device the per-element contraction over the
8 nodes runs on the TensorEngine: 16 elements per round are packed as 8x8
blocks on the diagonal of the stationary operand (built on the VectorEngine
as broadcast*mask in one fp16 2x op), and tile_position matmuls compute
    dd[(el,b,n), (g,l)] = sum_k disp[el][k,(b,n)] * shpdx[el][(g,l),k]
into PSUM.  Banks alternate between 4x(32x32) and 2x(64x64) subarray
splits so the VectorE (mask build, cost ~ W columns) and TensorE (cost ~
output columns) loads average out below the DMA roofline.  Full PSUM banks
(28 rounds = 448 elements) are evicted once as fp16 by the ScalarEngine and
DMA'd out; the host unpacks dd and combines the strain components.
"""

import sys

import numpy as np

sys.path.insert(0, "/opt/trn_rl_repo")

import concourse.bacc as bacc
import concourse.mybir as mybir
import concourse.tile as tile
from concourse.bass_utils import run_bass_kernel_spmd

B = 4
N_NODES = 1_000_000
N_ELEM = 500_000
N_GP = 9
N_EN = 8
N_CORES = 8

P = 128
NR = 28                    # rounds (16 elements each) per PSUM bank
EPB = 16 * NR              # elements per bank = 448
NBANK = 140                # banks per core
E_CORE = N_ELEM // N_CORES      # 62500
E_PAD = NBANK * EPB             # 62720
G = 10                     # banks per input DMA group
EG = 4                     # banks per output DMA group
NBG = NBANK // G           # 14
NEVG = NBANK // EG         # 35


def _is_sub2(bk: int) -> bool:
    return bk % 5 < 2


_compiled = None


def _build_program():
    nc = bacc.Bacc("TRN2", target_bir_lowering=False, debug=False)
    f16 = mybir.dt.float16
    f32 = mybir.dt.float32

    # disp, dense:  [bg, p=(grp,el,k), gi, r, (b,n)]
    d_d = nc.dram_tensor("d_in", [NBG, P, G, NR, 8], f16, kind="ExternalInput").ap()
    # shpdx, rhs-ready: [bg, p=(grp,el,k), gi, r, (g,l)]
    s_d = nc.dram_tensor("s_in", [NBG, P, G, NR, 18], f16, kind="ExternalInput").ap()
    # block-diag masks, packed: cols 0:32 sub4, cols 32:96 sub2
    m_d = nc.dram_tensor("mask", [P, 96], f16, kind="ExternalInput").ap()
    # zero ctx-index vector for the writeback stores
    z_d = nc.dram_tensor("zidx", [P, 1], mybir.dt.int32, kind="ExternalInput").ap()
    # dd out: [ev, 1, p=(grp, el', b, n), 1, (eslot, r, (g,l)) padded to 2048]
    o_d = nc.dram_tensor("out", [NEVG, 1, P, 1, 2048], f16, kind="ExternalOutput").ap()

    with tile.TileContext(nc) as tc:
        with (
            tc.tile_pool(name="const", bufs=1) as const_pool,
            tc.tile_pool(name="io", bufs=3) as io_pool,
            tc.tile_pool(name="w", bufs=4) as w_pool,
            tc.tile_pool(name="ps", bufs=8, space="PSUM") as ps_pool,
            tc.tile_pool(name="ev", bufs=20) as ev_pool,
        ):
            Mt = const_pool.tile([P, 96], f16, tag="M")
            nc.sync.dma_start(out=Mt[:], in_=m_d)
            Zt = const_pool.tile([P, 1], mybir.dt.int32, tag="Z")
            nc.sync.dma_start(out=Zt[:], in_=z_d)
            Mv = Mt[:, :32].rearrange("p (e b) -> p e b", e=4)
            M2v = Mt[:, 32:].rearrange("p (e b) -> p e b", e=8)

            ev = None
            for bg in range(NBG):
                D = io_pool.tile([P, G * NR * 8], f16, tag="D")
                S = io_pool.tile([P, G * NR * 18], f16, tag="S")
                nc.sync.dma_start(out=D[:], in_=d_d[bg].rearrange("p g r b -> p (g r b)"))
                nc.sync.dma_start(out=S[:], in_=s_d[bg].rearrange("p g r b -> p (g r b)"))

                for gi in range(G):
                    bk = bg * G + gi
                    if bk % EG == 0:
                        ev = ev_pool.tile([P, 2048], f16, tag="ev")
                        nc.vector.memset(ev[:, EG * NR * 18:], 0.0)

                    Dv = D[:].rearrange("p (g r b) -> p g r b", g=G, r=NR)
                    ps = ps_pool.tile([P, 512], f32, tag="ps")

                    if _is_sub2(bk):
                        # W[p, (r, el', bn)] = D[p, (gi, r, bn)] * M2[p, (el', bn)]
                        W = w_pool.tile([P, NR * 64], f16, tag="W2")
                        Wv = W[:].rearrange("p (r e b) -> p r e b", r=NR, e=8)
                        Db = Dv[:, gi, :, None, :].to_broadcast([P, NR, 8, 8])
                        Mb = M2v[:, None, :, :].to_broadcast([P, NR, 8, 8])
                        nc.vector.tensor_tensor(out=Wv, in0=Db, in1=Mb,
                                                op=mybir.AluOpType.mult)
                        for r in range(NR):
                            for h in range(2):
                                pr = slice(64 * h, 64 * h + 64)
                                nc.tensor.matmul(
                                    out=ps[pr, r * 18:(r + 1) * 18],
                                    lhsT=W[pr, r * 64:(r + 1) * 64],
                                    rhs=S[pr, (gi * NR + r) * 18:(gi * NR + r + 1) * 18],
                                    start=True, stop=True,
                                    tile_position=(64 * h, 64 * h),
                                )
                    else:
                        W = w_pool.tile([P, NR * 32], f16, tag="W")
                        Wv = W[:].rearrange("p (r e b) -> p r e b", r=NR, e=4)
                        Db = Dv[:, gi, :, None, :].to_broadcast([P, NR, 4, 8])
                        Mb = Mv[:, None, :, :].to_broadcast([P, NR, 4, 8])
                        nc.vector.tensor_tensor(out=Wv, in0=Db, in1=Mb,
                                                op=mybir.AluOpType.mult)
                        for r in range(NR):
                            for i in range(4):
                                pr = slice(32 * i, 32 * i + 32)
                                nc.tensor.matmul(
                                    out=ps[pr, r * 18:(r + 1) * 18],
                                    lhsT=W[pr, r * 32:(r + 1) * 32],
                                    rhs=S[pr, (gi * NR + r) * 18:(gi * NR + r + 1) * 18],
                                    start=True, stop=True,
                                    tile_position=(32 * i, 32 * i),
                                )

                    eslot = bk % EG
                    nc.scalar.copy(
                        out=ev[:, eslot * NR * 18:(eslot + 1) * NR * 18],
                        in_=ps[:, :NR * 18])

                    if eslot == EG - 1:
                        nc.gpsimd.kv_writeback(
                            out_ap=o_d[bk // EG],
                            in_ap=ev[:].rearrange("p (a b n) -> p a b n",
                                                  a=1, b=1),
                            ctx_idxs_ap=Zt[:])

    nc.compile()
    return nc


def _get_program():
    global _compiled
    if _compiled is None:
        _compiled = _build_program()
    return _compiled


def _make_masks():
    m = np.zeros((P, 96), np.float16)
    for p in range(P):
        el4 = (p % 32) // 8
        m[p, el4 * 8:(el4 + 1) * 8] = 1.0
        el8 = (p % 64) // 8
        m[p, 32 + el8 * 8:32 + (el8 + 1) * 8] = 1.0
    return m


_SUB2_BANKS = np.array([bk for bk in range(NBANK) if _is_sub2(bk)])
_SUB4_BANKS = np.array([bk for bk in range(NBANK) if not _is_sub2(bk)])


def _marshal_core(inputs_f16: np.ndarray, shpdx: np.ndarray,
                  elem_nodes: np.ndarray, c: int):
    """Build the d_in / s_in arrays for core c."""
    sl = slice(c * E_CORE, (c + 1) * E_CORE)
    en = elem_nodes[sl]                                   # [E, 8]
    disp = inputs_f16[:, en]                              # [B, E, 8, 2] f16
    # -> [E, k, (b, n)]
    dispc = np.ascontiguousarray(disp.transpose(1, 2, 0, 3)).reshape(E_CORE, 8, 8)
    dpad = np.zeros((E_PAD, 8, 8), np.float16)
    dpad[:E_CORE] = dispc
    spad = np.zeros((E_PAD, N_GP, 8, 2), np.float16)
    spad[:E_CORE] = shpdx[sl].astype(np.float16)

    # e = bk*EPB + r*16 + grp*per + el ; partition p = grp*per*8 + el*8 + k
    d_all = np.empty((NBANK, P, NR, 8), np.float16)
    s_all = np.empty((NBANK, P, NR, 18), np.float16)
    db = dpad.reshape(NBANK, NR, 16, 8, 8)                # bk r sub k bn
    sb = spad.reshape(NBANK, NR, 16, N_GP, 8, 2)          # bk r sub g k l
    for banks, ngrp, per in ((_SUB2_BANKS, 2, 8), (_SUB4_BANKS, 4, 4)):
        dv = db[banks].reshape(len(banks), NR, ngrp, per, 8, 8)
        dv = dv.transpose(0, 2, 3, 4, 1, 5)               # bk grp el k r bn
        d_all[banks] = dv.reshape(len(banks), P, NR, 8)
        sv = sb[banks].reshape(len(banks), NR, ngrp, per, N_GP, 8, 2)
        sv = sv.transpose(0, 2, 3, 5, 1, 4, 6)            # bk grp el k r g l
        s_all[banks] = sv.reshape(len(banks), P, NR, 18)

    d_in = np.ascontiguousarray(
        d_all.reshape(NBG, G, P, NR, 8).transpose(0, 2, 1, 3, 4))
    s_in = np.ascontiguousarray(
        s_all.reshape(NBG, G, P, NR, 18).transpose(0, 2, 1, 3, 4))
    return d_in, s_in


def _decode_core(o: np.ndarray) -> np.ndarray:
    """o: [NEVG, P, EG, NR, 18] fp16 -> strains [B, E_CORE*9, 3] f32."""
    # out partition p = grp*per*8 + el'*8 + (b*2+n)
    o = o[:, 0, :, 0, :EG * NR * 18]
    ob = o.reshape(NEVG, P, EG, NR, N_GP, 2).transpose(0, 2, 1, 3, 4, 5)
    ob = ob.reshape(NBANK, P, NR, N_GP, 2)                # bk p r g l
    dd = np.empty((B, E_PAD, N_GP, 2, 2), np.float16)
    ddv = dd.reshape(B, NBANK, NR, 16, N_GP, 2, 2)
    for banks, ngrp, per in ((_SUB2_BANKS, 2, 8), (_SUB4_BANKS, 4, 4)):
        ov = ob[banks].reshape(len(banks), ngrp, per, B, 2, NR, N_GP, 2)
        # -> b bk r (grp el) g n l
        ov = ov.transpose(3, 0, 5, 1, 2, 6, 4, 7)
        ddv[:, banks] = ov.reshape(B, len(banks), NR, 16, N_GP, 2, 2)
    dd = dd[:, :E_CORE].astype(np.float32)
    e_xx = dd[..., 0, 0]
    e_yy = dd[..., 1, 1]
    e_xy = 0.5 * (dd[..., 0, 1] + dd[..., 1, 0])
    e = np.stack([e_xx, e_yy, e_xy], axis=-1)             # [B, E, 9, 3]
    return e.reshape(B, E_CORE * N_GP, 3)


def kernel(inputs, shpdx, elem_nodes, _want_trace=False):
    nc = _get_program()

    inputs_f16 = inputs.astype(np.float16)
    mask = _make_masks()
    in_maps = []
    for c in range(N_CORES):
        d_in, s_in = _marshal_core(inputs_f16, shpdx, elem_nodes, c)
        in_maps.append({"d_in": d_in, "s_in": s_in, "mask": mask,
                        "zidx": np.zeros((P, 1), np.int32)})

    core_ids = list(range(N_CORES))
    res = run_bass_kernel_spmd(nc, in_maps, core_ids, trace=_want_trace)

    outs = []
    for c in range(N_CORES):
        outs.append(_decode_core(np.asarray(res.results[c]["out"])))
    full = np.concatenate(outs, axis=1)                   # [B, N_ELEM*9, 3]
    if _want_trace:
        return full, res
    return full
